# revision 2
# baseline (speedup 1.0000x reference)
"""Trainium2 Bass kernel for batched masked attention.

Problem: q,k,v [16, 2048, 256] f32, mask [16, 2048, 2048] int32.
  scores = (q @ k^T) / 16
  scores = where(mask == 0, 0.0, scores)      # NOT -inf
  att    = softmax(scores, axis=-1)
  att    = 0 if mask.sum() == 0 (handled host-side)
  out    = att @ v
Sharding: batch dim across 8 NeuronCores (2 batches per core).

Math restructure vs the f32r baseline: with att = exp(s~) where s~ is the
masked/scaled score, note att = t + 1 where t := (exp(s/16) - 1) * m
(masked positions contribute exp(0) = 1). Then
  att @ [v|1|1] = t @ [v|1|1] + [colsum(v) | S | S]
so the mask is applied POST-exp on cheap bf16 SBUF data and the +1
correction rides a 17th all-ones contraction block whose moving operand is
the host-precomputed column-sum of v (replicated over partitions).

Engine plan per 512-query chunk (cost-model rates):
  mm1 (PE): scoresT = k^T q via fp8e4 DoubleRow (0.5 cyc/row), with
    q = q_hi + q_lo, k = k_hi + k_lo hi/lo fp8 splits; 3 DR matmuls
    (hh, lh, hl) per 128-key block replace 2 f32r matmuls: 5.1us
  ACT: e = exp(s * 1/16) PSUM f32 -> SBUF bf16 (scale folded in): 10.1us
  DVE: t = (e - 1) * mask_u8 in-place STT, + 1/Z scale epilogue: 11.5us
  mm2 (PE): t @ [v|1|1] in bf16 (1 cyc/row) + ones@colsum block: 7.3us
PE bound ~12.4us/chunk vs 13.7 for the f32r baseline; fp8/bf16/u8 operands
also halve HBM traffic (16MB/core vs 24MB).
"""

import sys

if "/opt/trn_rl_repo" not in sys.path:
    sys.path.insert(0, "/opt/trn_rl_repo")

from contextlib import ExitStack

import numpy as np
import ml_dtypes

import concourse.mybir as mybir
import concourse.tile as tile
from concourse import bacc
from concourse.bass_utils import run_bass_kernel_spmd

B, S, D = 16, 2048, 256
NCORES = 8
BPC = B // NCORES  # batches per core
P = 128
QT = S // P        # 16 key blocks of 128
IC = S // 512      # 4 query chunks of 512
KC = D // P        # 2 contraction chunks of 128
SCALE = 1.0 / 16.0  # 1/sqrt(D), folded into the exp activation

F32 = mybir.dt.float32
BF16 = mybir.dt.bfloat16
FP8 = mybir.dt.float8e4
U8 = mybir.dt.uint8
E4M3 = ml_dtypes.float8_e4m3
NP_BF16 = ml_dtypes.bfloat16

DR = mybir.MatmulPerfMode.DoubleRow

# mask-STT pairs offloaded from DVE to the gpsimd (Pool) engine
STT_POOL_JPS = ()


def build_program(reps=1):
    nc = bacc.Bacc("TRN2", target_bir_lowering=False, debug=False)
    qhd = nc.dram_tensor("qh", [BPC, P, KC, S], FP8, kind="ExternalInput").ap()
    qld = nc.dram_tensor("ql", [BPC, P, KC, S], FP8, kind="ExternalInput").ap()
    khd = nc.dram_tensor("kh", [BPC, P, KC, S], FP8, kind="ExternalInput").ap()
    kld = nc.dram_tensor("kl", [BPC, P, KC, S], FP8, kind="ExternalInput").ap()
    vpd = nc.dram_tensor("vp", [BPC, P, QT, D + 2], BF16, kind="ExternalInput").ap()
    csd = nc.dram_tensor("cs", [BPC, P, D + 2], BF16, kind="ExternalInput").ap()
    m8d = nc.dram_tensor("mask8", [BPC, IC, P, QT, 512], U8, kind="ExternalInput").ap()
    out = nc.dram_tensor("out", [BPC, S, D], BF16, kind="ExternalOutput").ap()

    with tile.TileContext(nc) as tc, ExitStack() as ctx:
        kh_pool = ctx.enter_context(tc.tile_pool(name="kh", bufs=2))
        kl_pool = ctx.enter_context(tc.tile_pool(name="kl", bufs=2))
        qh_pool = ctx.enter_context(tc.tile_pool(name="qh", bufs=2))
        ql_pool = ctx.enter_context(tc.tile_pool(name="ql", bufs=2))
        vp_pool = ctx.enter_context(tc.tile_pool(name="vp", bufs=2))
        cs_pool = ctx.enter_context(tc.tile_pool(name="cs", bufs=2))
        mask_pool = ctx.enter_context(tc.tile_pool(name="maskp", bufs=3))
        att_pool = ctx.enter_context(tc.tile_pool(name="att", bufs=2))
        osb_pool = ctx.enter_context(tc.tile_pool(name="osb", bufs=4))
        rec_pool = ctx.enter_context(tc.tile_pool(name="rec", bufs=4))
        one_pool = ctx.enter_context(tc.tile_pool(name="onep", bufs=1))
        # ps_s tiles span 2 PSUM banks (a PAIR of key blocks) so one exp and
        # one STT cover 1024 columns, halving their per-op overhead
        ps_s = ctx.enter_context(tc.tile_pool(name="ps_s", bufs=3, space="PSUM"))
        ps_out = ctx.enter_context(tc.tile_pool(name="ps_out", bufs=2, space="PSUM"))

        def build_inputs(b):
            # chunked loads so each mm1 only waits for the slices it reads
            kh = kh_pool.tile([P, KC, S], FP8, tag="kh")
            kl = kl_pool.tile([P, KC, S], FP8, tag="kl")
            qh = qh_pool.tile([P, KC, S], FP8, tag="qh")
            ql = ql_pool.tile([P, KC, S], FP8, tag="ql")
            for jb in range(4):
                sl = slice(jb * P, (jb + 1) * P)
                nc.gpsimd.dma_start(kh[:, :, sl], khd[b][:, :, sl])
                nc.gpsimd.dma_start(kl[:, :, sl], kld[b][:, :, sl])
            nc.gpsimd.dma_start(qh[:, :, :512], qhd[b][:, :, :512])
            nc.gpsimd.dma_start(ql[:, :, :512], qld[b][:, :, :512])
            for c in range(1, IC):
                sl = slice(c * 512, (c + 1) * 512)
                nc.gpsimd.dma_start(kh[:, :, sl], khd[b][:, :, sl])
                nc.gpsimd.dma_start(kl[:, :, sl], kld[b][:, :, sl])
            for c in range(1, IC):
                sl = slice(c * 512, (c + 1) * 512)
                nc.gpsimd.dma_start(qh[:, :, sl], qhd[b][:, :, sl])
                nc.gpsimd.dma_start(ql[:, :, sl], qld[b][:, :, sl])
            vp = vp_pool.tile([P, QT, D + 2], BF16, tag="vp")
            nc.gpsimd.dma_start(vp[:], vpd[b])
            cs = cs_pool.tile([P, D + 2], BF16, tag="cs")
            nc.gpsimd.dma_start(cs[:], csd[b])
            return kh, kl, qh, ql, vp, cs

        def mm1_pair(ic, jp, kh, kl, qh, ql, mt, att):
            """scoresT + exp + mask for key blocks 2jp, 2jp+1 of chunk ic."""
            qsl = slice(ic * 512, (ic + 1) * 512)
            ps = ps_s.tile([P, 1024], F32, tag="score")
            for half in range(2):
                jb = 2 * jp + half
                ksl = slice(jb * P, (jb + 1) * P)
                osl = slice(half * 512, (half + 1) * 512)
                # q @ k ~= qh@kh + ql@kh + qh@kl, each a DoubleRow matmul
                # contracting both 128-chunks of D at 0.5 cyc/row
                nc.tensor.matmul(
                    ps[:, osl], lhsT=kh[:, :, ksl], rhs=qh[:, :, qsl],
                    start=True, stop=False, perf_mode=DR,
                )
                nc.tensor.matmul(
                    ps[:, osl], lhsT=kl[:, :, ksl], rhs=qh[:, :, qsl],
                    start=False, stop=False, perf_mode=DR,
                )
                nc.tensor.matmul(
                    ps[:, osl], lhsT=kh[:, :, ksl], rhs=ql[:, :, qsl],
                    start=False, stop=True, perf_mode=DR,
                )
            asl = att[:, 2 * jp : 2 * jp + 2, :]
            nc.scalar.activation(
                asl, ps[:], mybir.ActivationFunctionType.Exp, scale=SCALE
            )
            eng = nc.gpsimd if jp in STT_POOL_JPS else nc.vector
            eng.scalar_tensor_tensor(
                out=asl, in0=asl, scalar=-1.0, in1=mt[:, 2 * jp : 2 * jp + 2, :],
                op0=mybir.AluOpType.add, op1=mybir.AluOpType.mult,
            )

        def mm2_group(b, ic, att, vp, cs, ones, iq):
            """t.T @ [v|1|1] + colsum + normalize + store, query tile iq."""
            po = ps_out.tile([P, D + 2], F32, tag="ps_out")
            isl = slice(iq * P, (iq + 1) * P)
            for jb in range(QT):
                nc.tensor.matmul(
                    po[:], lhsT=att[:, jb, isl], rhs=vp[:, jb, :],
                    start=(jb == 0), stop=False,
                )
            # 17th block: ones/128 lhsT sums cs over partitions -> +colsum(v)
            # in cols 0..255 and +S in the Z column
            nc.tensor.matmul(po[:], lhsT=ones[:], rhs=cs[:], start=False, stop=True)
            rec = rec_pool.tile([P, 1], F32, tag="rec")
            nc.vector.reciprocal(rec[:], po[:, D : D + 1])
            osb = osb_pool.tile([P, D], BF16, tag="osb")
            nc.vector.tensor_scalar_mul(osb[:], po[:, :D], rec[:])
            it = ic * 4 + iq
            nc.sync.dma_start(out[b, it * P : (it + 1) * P, :], osb[:])

        batches = [b for _ in range(reps) for b in range(BPC)]
        # PE warm-up: dummy matmuls during the initial DMA wait so the clock
        # gate is at 2.4 GHz when real work arrives
        warm = one_pool.tile([P, 512], BF16, tag="warm")
        nc.gpsimd.memset(warm[:], 0.0)
        ones = one_pool.tile([P, P], BF16, tag="ones")
        nc.gpsimd.memset(ones[:], 1.0 / P)
        for i in range(4):
            wp = ps_out.tile([P, 512], F32, tag="ps_out")
            nc.tensor.matmul(
                wp[:], lhsT=warm[:, :P], rhs=warm[:], start=True, stop=True
            )
        inputs = {0: build_inputs(batches[0])}
        pending = None
        for idx, b in enumerate(batches):
            kh, kl, qh, ql, vp, cs = inputs.pop(idx)
            for ic in range(IC):
                mt = mask_pool.tile([P, QT, 512], U8, tag="maskt")
                if idx == 0 and ic == 0:
                    # split the first mask load so the STT on key block 0
                    # starts after 256KB instead of 1MB
                    for g4 in range(4):
                        nc.sync.dma_start(
                            mt[:, g4 * 4 : (g4 + 1) * 4, :],
                            m8d[b, ic, :, g4 * 4 : (g4 + 1) * 4, :],
                        )
                else:
                    nc.sync.dma_start(mt[:], m8d[b, ic])
                att = att_pool.tile([P, QT, 512], BF16, tag="att")
                for g in range(4):
                    mm1_pair(ic, 2 * g, kh, kl, qh, ql, mt, att)
                    mm1_pair(ic, 2 * g + 1, kh, kl, qh, ql, mt, att)
                    if pending is not None:
                        mm2_group(*pending, ones, iq=g)
                if ic == 1 and idx + 1 < len(batches):
                    inputs[idx + 1] = build_inputs(batches[idx + 1])
                pending = (b, ic, att, vp, cs)
        for g in range(4):
            mm2_group(*pending, ones, iq=g)

    nc.compile()
    return nc


def prep_inputs(q, k, v, mask):
    """Host-side layout prep; returns per-core in_maps."""
    q = np.asarray(q, dtype=np.float32)
    k = np.asarray(k, dtype=np.float32)
    v = np.asarray(v, dtype=np.float32)
    # [B, S, D] -> [B, P, KC, S]  (transposed, head-dim on partitions)
    qt = np.ascontiguousarray(
        q.transpose(0, 2, 1).reshape(B, KC, P, S).transpose(0, 2, 1, 3)
    )
    kt = np.ascontiguousarray(
        k.transpose(0, 2, 1).reshape(B, KC, P, S).transpose(0, 2, 1, 3)
    )
    # hi/lo fp8e4 splits: x ~= x_hi + x_lo with |err| ~ 2.5%^2
    qh = qt.astype(E4M3)
    ql = (qt - qh.astype(np.float32)).astype(E4M3)
    kh = kt.astype(E4M3)
    kl = (kt - kh.astype(np.float32)).astype(E4M3)
    # [B, S, D] -> [B, P, QT, D+2] bf16 with ones in the last two columns
    vp = np.ones((B, P, QT, D + 2), dtype=NP_BF16)
    vp[..., :D] = v.reshape(B, QT, P, D).transpose(0, 2, 1, 3).astype(NP_BF16)
    # column sums of [v|1|1], replicated across partitions, pre-divided by
    # nothing: the device lhsT carries the 1/128
    csv = np.full((B, D + 2), float(S), dtype=np.float32)
    csv[:, :D] = v.sum(axis=1)
    cs = np.broadcast_to(csv[:, None, :], (B, P, D + 2)).astype(NP_BF16)
    cs = np.ascontiguousarray(cs)
    # mask [B, S(query), S(key)] -> u8 tiles [B, IC, P(key), QT, 512(query)]
    m8 = np.ascontiguousarray(
        (np.asarray(mask) != 0)
        .astype(np.uint8)
        .reshape(B, IC, 512, QT, P)
        .transpose(0, 1, 4, 3, 2)
    )
    return [
        {
            "qh": qh[c * BPC : (c + 1) * BPC],
            "ql": ql[c * BPC : (c + 1) * BPC],
            "kh": kh[c * BPC : (c + 1) * BPC],
            "kl": kl[c * BPC : (c + 1) * BPC],
            "vp": vp[c * BPC : (c + 1) * BPC],
            "cs": cs[c * BPC : (c + 1) * BPC],
            "mask8": m8[c * BPC : (c + 1) * BPC],
        }
        for c in range(NCORES)
    ]


_NC_CACHE = None


def _get_program():
    global _NC_CACHE
    if _NC_CACHE is None:
        _NC_CACHE = build_program()
    return _NC_CACHE


def kernel(q, k, v, mask):
    mask = np.asarray(mask)
    if mask.sum() == 0:
        return np.zeros((B, S, D), dtype=np.float32)
    nc = _get_program()
    in_maps = prep_inputs(q, k, v, mask)
    res = run_bass_kernel_spmd(nc, in_maps, list(range(NCORES)))
    got = np.concatenate([res.results[c]["out"] for c in range(NCORES)], axis=0)
    return np.asarray(got).astype(np.float32)


# revision 28
# speedup vs baseline: 1.1923x; 1.1923x over previous
"""Trainium2 Bass kernel for batched masked attention.

Problem: q,k,v [16, 2048, 256] f32, mask [16, 2048, 2048] int32.
  scores = (q @ k^T) / 16
  scores = where(mask == 0, 0.0, scores)      # NOT -inf
  att    = softmax(scores, axis=-1)
  att    = 0 if mask.sum() == 0 (handled host-side)
  out    = att @ v
Sharding: batch dim across 8 NeuronCores (2 batches per core).

Math restructure vs the f32r baseline: with att = exp(s~) where s~ is the
masked/scaled score, note att = t + 1 where t := (exp(s/16) - 1) * m
(masked positions contribute exp(0) = 1). Then
  att @ [v|1|1] = t @ [v|1|1] + [colsum(v) | S | S]
so the mask is applied POST-exp on cheap bf16 SBUF data and the +1
correction rides a 17th all-ones contraction block whose moving operand is
the host-precomputed column-sum of v (replicated over partitions).

Engine plan per 512-query chunk (cost-model rates):
  mm1 (PE): scoresT = k^T q via fp8e4 DoubleRow (0.5 cyc/row), with
    q = q_hi + q_lo, k = k_hi + k_lo hi/lo fp8 splits; 3 DR matmuls
    (hh, lh, hl) per 128-key block replace 2 f32r matmuls: 5.1us
  ACT: e = exp(s * 1/16) PSUM f32 -> SBUF bf16 (scale folded in): 10.1us
  DVE: t = (e - 1) * mask_u8 in-place STT, + 1/Z scale epilogue: 11.5us
  mm2 (PE): t @ [v|1|1] in bf16 (1 cyc/row) + ones@colsum block: 7.3us
PE bound ~12.4us/chunk vs 13.7 for the f32r baseline; fp8/bf16/u8 operands
also halve HBM traffic (16MB/core vs 24MB).
"""

import sys

if "/opt/trn_rl_repo" not in sys.path:
    sys.path.insert(0, "/opt/trn_rl_repo")

from contextlib import ExitStack

import numpy as np
import ml_dtypes

import concourse.mybir as mybir
import concourse.tile as tile
from concourse import bacc
from concourse.bass_utils import run_bass_kernel_spmd

B, S, D = 16, 2048, 256
NCORES = 8
BPC = B // NCORES  # batches per core
P = 128
QT = S // P        # 16 key blocks of 128
IC = S // 512      # 4 query chunks of 512
KC = D // P        # 2 contraction chunks of 128
SCALE = 1.0 / 16.0  # 1/sqrt(D), folded into the exp activation

F32 = mybir.dt.float32
BF16 = mybir.dt.bfloat16
FP8 = mybir.dt.float8e4
U8 = mybir.dt.uint8
E4M3 = ml_dtypes.float8_e4m3
NP_BF16 = ml_dtypes.bfloat16

DR = mybir.MatmulPerfMode.DoubleRow

# mask-STT pairs offloaded from DVE to the gpsimd (Pool) engine
STT_POOL_JPS = ()


def build_program(reps=1):
    nc = bacc.Bacc("TRN2", target_bir_lowering=False, debug=False)
    # kh/kl/qh/ql packed in one tensor so prologue loads are few big DMAs
    qkd = nc.dram_tensor("qk8", [BPC, P, 4, KC, S], FP8, kind="ExternalInput").ap()
    vpd = nc.dram_tensor("vp", [BPC, P, QT, D + 2], BF16, kind="ExternalInput").ap()
    csd = nc.dram_tensor("cs", [BPC, P, D + 2], BF16, kind="ExternalInput").ap()
    m8d = nc.dram_tensor("mask8", [BPC, IC, P, QT, 512], U8, kind="ExternalInput").ap()
    out = nc.dram_tensor("out", [BPC, S, D], BF16, kind="ExternalOutput").ap()

    with tile.TileContext(nc) as tc, ExitStack() as ctx:
        qk_pool = ctx.enter_context(tc.tile_pool(name="qk", bufs=2))
        vp_pool = ctx.enter_context(tc.tile_pool(name="vp", bufs=2))
        cs_pool = ctx.enter_context(tc.tile_pool(name="cs", bufs=2))
        mask_pool = ctx.enter_context(tc.tile_pool(name="maskp", bufs=3))
        att_pool = ctx.enter_context(tc.tile_pool(name="att", bufs=2))
        osb_pool = ctx.enter_context(tc.tile_pool(name="osb", bufs=4))
        rec_pool = ctx.enter_context(tc.tile_pool(name="rec", bufs=4))
        one_pool = ctx.enter_context(tc.tile_pool(name="onep", bufs=1))
        # ps_s tiles span 2 PSUM banks (a PAIR of key blocks) so one exp and
        # one STT cover 1024 columns, halving their per-op overhead
        ps_s = ctx.enter_context(tc.tile_pool(name="ps_s", bufs=3, space="PSUM"))
        ps_out = ctx.enter_context(tc.tile_pool(name="ps_out", bufs=2, space="PSUM"))

        def build_inputs(b):
            # first-needed slices ride the fast SP HWDGE ring (sub-us
            # trigger); the bulk rides the gpsimd SWDGE ring (~1us per
            # dma_start of Pool time), split so no single transfer hogs the
            # shared DMA engines ahead of the mask stream
            qk = qk_pool.tile([P, 4, KC, S], FP8, tag="qk")
            kh, kl, qh, ql = (qk[:, i] for i in range(4))
            # kh+kl full (every key block feeds every chunk) + first q chunk
            nc.sync.dma_start(qk[:, 0:2], qkd[b][:, 0:2])
            nc.sync.dma_start(qk[:, 2:4, :, :512], qkd[b][:, 2:4, :, :512])
            nc.gpsimd.dma_start(
                qk[:, 2:4, :, 512:1024], qkd[b][:, 2:4, :, 512:1024]
            )
            vp = vp_pool.tile([P, QT, D + 2], BF16, tag="vp")
            nc.gpsimd.dma_start(vp[:], vpd[b])
            cs = cs_pool.tile([P, D + 2], BF16, tag="cs")
            nc.gpsimd.dma_start(cs[:], csd[b])
            nc.gpsimd.dma_start(
                qk[:, 2:4, :, 1024:], qkd[b][:, 2:4, :, 1024:]
            )
            return kh, kl, qh, ql, vp, cs

        def load_mask(b, ic, first=False):
            """Mask loads own the sync ring so they trigger immediately and
            prefetch ahead; out-stores ride the ACT HWDGE instead."""
            mt = mask_pool.tile([P, QT, 512], U8, tag="maskt")
            if first:
                # split the first load so the STT on key block 0 starts
                # after 256KB instead of 2MB
                for lo, hi in ((0, 4), (4, 8), (8, 16)):
                    nc.sync.dma_start(
                        mt[:, lo:hi, :], m8d[b, ic, :, lo:hi, :]
                    )
            else:
                nc.sync.dma_start(mt[:], m8d[b, ic])
            return mt

        def mm1_pair(ic, jp, kh, kl, qh, ql, mt, att):
            """scoresT + exp + mask for key blocks 2jp, 2jp+1 of chunk ic."""
            qsl = slice(ic * 512, (ic + 1) * 512)
            ps = ps_s.tile([P, 1024], F32, tag="score")
            for half in range(2):
                jb = 2 * jp + half
                ksl = slice(jb * P, (jb + 1) * P)
                osl = slice(half * 512, (half + 1) * 512)
                # q @ k ~= qh@kh + ql@kh + qh@kl, each a DoubleRow matmul
                # contracting both 128-chunks of D at 0.5 cyc/row
                nc.tensor.matmul(
                    ps[:, osl], lhsT=kh[:, :, ksl], rhs=qh[:, :, qsl],
                    start=True, stop=False, perf_mode=DR,
                )
                nc.tensor.matmul(
                    ps[:, osl], lhsT=kl[:, :, ksl], rhs=qh[:, :, qsl],
                    start=False, stop=False, perf_mode=DR,
                )
                nc.tensor.matmul(
                    ps[:, osl], lhsT=kh[:, :, ksl], rhs=ql[:, :, qsl],
                    start=False, stop=True, perf_mode=DR,
                )
            asl = att[:, 2 * jp : 2 * jp + 2, :]
            nc.scalar.activation(
                asl, ps[:], mybir.ActivationFunctionType.Exp, scale=SCALE
            )
            eng = nc.gpsimd if jp in STT_POOL_JPS else nc.vector
            eng.scalar_tensor_tensor(
                out=asl, in0=asl, scalar=-1.0, in1=mt[:, 2 * jp : 2 * jp + 2, :],
                op0=mybir.AluOpType.add, op1=mybir.AluOpType.mult,
            )

        def mm2_group(b, ic, att, vp, cs, ones, iq, last=False):
            """t.T @ [v|1|1] + colsum + normalize + store, query tile iq."""
            po = ps_out.tile([P, D + 2], F32, tag="ps_out")
            isl = slice(iq * P, (iq + 1) * P)
            for jb in range(QT):
                nc.tensor.matmul(
                    po[:], lhsT=att[:, jb, isl], rhs=vp[:, jb, :],
                    start=(jb == 0), stop=False,
                )
            # 17th block: ones/128 lhsT sums cs over partitions -> +colsum(v)
            # in cols 0..255 and +S in the Z column
            nc.tensor.matmul(po[:], lhsT=ones[:], rhs=cs[:], start=False, stop=True)
            rec = rec_pool.tile([P, 1], F32, tag="rec")
            nc.vector.reciprocal(rec[:], po[:, D : D + 1])
            osb = osb_pool.tile([P, D], BF16, tag="osb")
            if iq % 2 == 0:
                nc.scalar.activation(
                    osb[:], po[:, :D],
                    mybir.ActivationFunctionType.Copy, scale=rec[:],
                )
            else:
                nc.vector.tensor_scalar_mul(osb[:], po[:, :D], rec[:])
            it = ic * 4 + iq
            # the final chunk's stores ride the then-idle SP ring (no masks
            # left to block); earlier stores stay off it
            eng = nc.sync if last else nc.scalar
            eng.dma_start(out[b, it * P : (it + 1) * P, :], osb[:])

        batches = [b for _ in range(reps) for b in range(BPC)]
        # PE warm-up: dummy matmuls during the initial DMA wait so the clock
        # gate is at 2.4 GHz when real work arrives
        warm = one_pool.tile([P, 256], BF16, tag="warm")
        nc.vector.memset(warm[:], 0.0)
        ones = one_pool.tile([P, P], BF16, tag="ones")
        nc.vector.memset(ones[:], 1.0 / P)
        for i in range(12):
            wp = ps_out.tile([P, 256], F32, tag="ps_out")
            nc.tensor.matmul(
                wp[:], lhsT=warm[:, :P], rhs=warm[:], start=True, stop=True
            )
        inputs = {0: build_inputs(batches[0])}
        masks = {(0, 0): load_mask(batches[0], 0, first=True)}
        pending = None

        def get_mask(idx, ic):
            if (idx, ic) not in masks:
                masks[(idx, ic)] = load_mask(batches[idx], ic)
            return masks.pop((idx, ic))

        def prefetch_mask(idx, ic):
            nxt = (idx + (ic + 1) // IC, (ic + 1) % IC)
            if nxt[0] < len(batches) and nxt not in masks:
                masks[nxt] = load_mask(batches[nxt[0]], nxt[1])

        for idx, b in enumerate(batches):
            kh, kl, qh, ql, vp, cs = inputs.pop(idx)
            for ic in range(IC):
                mt = get_mask(idx, ic)
                att = att_pool.tile([P, QT, 512], BF16, tag="att")
                for g in range(4):
                    mm1_pair(ic, 2 * g, kh, kl, qh, ql, mt, att)
                    mm1_pair(ic, 2 * g + 1, kh, kl, qh, ql, mt, att)
                    if pending is not None:
                        mm2_group(*pending, ones, iq=g)
                if ic == 1 and idx + 1 < len(batches):
                    inputs[idx + 1] = build_inputs(batches[idx + 1])
                prefetch_mask(idx, ic)
                pending = (b, ic, att, vp, cs)
        for g in range(4):
            mm2_group(*pending, ones, iq=g, last=True)

    nc.compile()
    return nc


def prep_inputs(q, k, v, mask):
    """Host-side layout prep; returns per-core in_maps."""
    q = np.asarray(q, dtype=np.float32)
    k = np.asarray(k, dtype=np.float32)
    v = np.asarray(v, dtype=np.float32)
    # [B, S, D] -> [B, P, KC, S]  (transposed, head-dim on partitions)
    qt = np.ascontiguousarray(
        q.transpose(0, 2, 1).reshape(B, KC, P, S).transpose(0, 2, 1, 3)
    )
    kt = np.ascontiguousarray(
        k.transpose(0, 2, 1).reshape(B, KC, P, S).transpose(0, 2, 1, 3)
    )
    # hi/lo fp8e4 splits: x ~= x_hi + x_lo with |err| ~ 2.5%^2
    qh = qt.astype(E4M3)
    ql = (qt - qh.astype(np.float32)).astype(E4M3)
    kh = kt.astype(E4M3)
    kl = (kt - kh.astype(np.float32)).astype(E4M3)
    # pack [B, P, 4, KC, S]: slots kh, kl, qh, ql
    qk8 = np.ascontiguousarray(
        np.stack((kh, kl, qh, ql), axis=1).transpose(0, 2, 1, 3, 4)
    )
    # [B, S, D] -> [B, P, QT, D+2] bf16 with ones in the last two columns
    vp = np.ones((B, P, QT, D + 2), dtype=NP_BF16)
    vp[..., :D] = v.reshape(B, QT, P, D).transpose(0, 2, 1, 3).astype(NP_BF16)
    # column sums of [v|1|1], replicated across partitions, pre-divided by
    # nothing: the device lhsT carries the 1/128
    csv = np.full((B, D + 2), float(S), dtype=np.float32)
    csv[:, :D] = v.sum(axis=1)
    cs = np.broadcast_to(csv[:, None, :], (B, P, D + 2)).astype(NP_BF16)
    cs = np.ascontiguousarray(cs)
    # mask [B, S(query), S(key)] -> u8 tiles [B, IC, P(key), QT, 512(query)]
    m8 = np.ascontiguousarray(
        (np.asarray(mask) != 0)
        .astype(np.uint8)
        .reshape(B, IC, 512, QT, P)
        .transpose(0, 1, 4, 3, 2)
    )
    return [
        {
            "qk8": qk8[c * BPC : (c + 1) * BPC],
            "vp": vp[c * BPC : (c + 1) * BPC],
            "cs": cs[c * BPC : (c + 1) * BPC],
            "mask8": m8[c * BPC : (c + 1) * BPC],
        }
        for c in range(NCORES)
    ]


_NC_CACHE = None


def _get_program():
    global _NC_CACHE
    if _NC_CACHE is None:
        _NC_CACHE = build_program()
    return _NC_CACHE


def kernel(q, k, v, mask):
    mask = np.asarray(mask)
    if mask.sum() == 0:
        return np.zeros((B, S, D), dtype=np.float32)
    nc = _get_program()
    in_maps = prep_inputs(q, k, v, mask)
    res = run_bass_kernel_spmd(nc, in_maps, list(range(NCORES)))
    got = np.concatenate([res.results[c]["out"] for c in range(NCORES)], axis=0)
    return np.asarray(got).astype(np.float32)


# revision 40
# speedup vs baseline: 1.2035x; 1.0094x over previous
"""Trainium2 Bass kernel for batched masked attention.

Problem: q,k,v [16, 2048, 256] f32, mask [16, 2048, 2048] int32.
  scores = (q @ k^T) / 16
  scores = where(mask == 0, 0.0, scores)      # NOT -inf
  att    = softmax(scores, axis=-1)
  att    = 0 if mask.sum() == 0 (handled host-side)
  out    = att @ v
Sharding: batch dim across 8 NeuronCores (2 batches per core).

Math restructure vs the f32r baseline: with att = exp(s~) where s~ is the
masked/scaled score, note att = t + 1 where t := (exp(s/16) - 1) * m
(masked positions contribute exp(0) = 1). Then
  att @ [v|1|1] = t @ [v|1|1] + [colsum(v) | S | S]
so the mask is applied POST-exp on cheap bf16 SBUF data and the +1
correction rides a 17th all-ones contraction block whose moving operand is
the host-precomputed column-sum of v (replicated over partitions).

Engine plan per 512-query chunk (cost-model rates):
  mm1 (PE): scoresT = k^T q via fp8e4 DoubleRow (0.5 cyc/row), with
    q = q_hi + q_lo, k = k_hi + k_lo hi/lo fp8 splits; 3 DR matmuls
    (hh, lh, hl) per 128-key block replace 2 f32r matmuls: 5.1us
  ACT: e = exp(s * 1/16) PSUM f32 -> SBUF bf16 (scale folded in): 10.1us
  DVE: t = (e - 1) * mask_u8 in-place STT, + 1/Z scale epilogue: 11.5us
  mm2 (PE): t @ [v|1|1] in bf16 (1 cyc/row) + ones@colsum block: 7.3us
PE bound ~12.4us/chunk vs 13.7 for the f32r baseline; fp8/bf16/u8 operands
also halve HBM traffic (16MB/core vs 24MB).
"""

import sys

if "/opt/trn_rl_repo" not in sys.path:
    sys.path.insert(0, "/opt/trn_rl_repo")

from contextlib import ExitStack

import numpy as np
import ml_dtypes

import concourse.mybir as mybir
import concourse.tile as tile
from concourse import bacc
from concourse.bass_utils import run_bass_kernel_spmd

B, S, D = 16, 2048, 256
NCORES = 8
BPC = B // NCORES  # batches per core
P = 128
QT = S // P        # 16 key blocks of 128
IC = S // 512      # 4 query chunks of 512
KC = D // P        # 2 contraction chunks of 128
SCALE = 1.0 / 16.0  # 1/sqrt(D), folded into the exp activation

F32 = mybir.dt.float32
BF16 = mybir.dt.bfloat16
FP8 = mybir.dt.float8e4
U8 = mybir.dt.uint8
E4M3 = ml_dtypes.float8_e4m3
NP_BF16 = ml_dtypes.bfloat16

DR = mybir.MatmulPerfMode.DoubleRow

# mask-STT pairs offloaded from DVE to the gpsimd (Pool) engine
STT_POOL_JPS = ()


def build_program(reps=1):
    nc = bacc.Bacc("TRN2", target_bir_lowering=False, debug=False)
    # kh/kl/qh/ql packed in one tensor so prologue loads are few big DMAs
    qkd = nc.dram_tensor("qk8", [BPC, P, 4, KC, S], FP8, kind="ExternalInput").ap()
    vpd = nc.dram_tensor("vp", [BPC, P, QT, D + 1], BF16, kind="ExternalInput").ap()
    csd = nc.dram_tensor("cs", [BPC, P, D + 1], BF16, kind="ExternalInput").ap()
    m8d = nc.dram_tensor("mask8", [BPC, IC, P, QT, 512], U8, kind="ExternalInput").ap()
    out = nc.dram_tensor("out", [BPC, S, D], BF16, kind="ExternalOutput").ap()

    with tile.TileContext(nc) as tc, ExitStack() as ctx:
        qk_pool = ctx.enter_context(tc.tile_pool(name="qk", bufs=2))
        vp_pool = ctx.enter_context(tc.tile_pool(name="vp", bufs=2))
        cs_pool = ctx.enter_context(tc.tile_pool(name="cs", bufs=2))
        mask_pool = ctx.enter_context(tc.tile_pool(name="maskp", bufs=3))
        att_pool = ctx.enter_context(tc.tile_pool(name="att", bufs=2))
        osb_pool = ctx.enter_context(tc.tile_pool(name="osb", bufs=4))
        rec_pool = ctx.enter_context(tc.tile_pool(name="rec", bufs=4))
        one_pool = ctx.enter_context(tc.tile_pool(name="onep", bufs=1))
        # ps_s tiles span 2 PSUM banks (a PAIR of key blocks) so one exp and
        # one STT cover 1024 columns, halving their per-op overhead
        ps_s = ctx.enter_context(tc.tile_pool(name="ps_s", bufs=3, space="PSUM"))
        ps_out = ctx.enter_context(tc.tile_pool(name="ps_out", bufs=2, space="PSUM"))

        def build_inputs(b, first=False):
            # first-needed slices ride the fast SP HWDGE ring (sub-us
            # trigger); the bulk rides the gpsimd SWDGE ring (~1us per
            # dma_start of Pool time), split so no single transfer hogs the
            # shared DMA engines ahead of the mask stream
            qk = qk_pool.tile([P, 4, KC, S], FP8, tag="qk")
            kh, kl, qh, ql = (qk[:, i] for i in range(4))
            # kh+kl full (every key block feeds every chunk) + first q chunk
            nc.sync.dma_start(qk[:, 0:2], qkd[b][:, 0:2])
            nc.sync.dma_start(qk[:, 2:4, :, :512], qkd[b][:, 2:4, :, :512])
            if first:
                # batch 0: chunk-1 q goes on the fast ring AFTER the first
                # mask pieces (emitted by the caller, in consumption order)
                deferred.append(
                    lambda: nc.sync.dma_start(
                        qk[:, 2:4, :, 512:1024], qkd[b][:, 2:4, :, 512:1024]
                    )
                )
            else:
                nc.gpsimd.dma_start(
                    qk[:, 2:4, :, 512:1024], qkd[b][:, 2:4, :, 512:1024]
                )
            vp = vp_pool.tile([P, QT, D + 1], BF16, tag="vp")
            nc.gpsimd.dma_start(vp[:], vpd[b])
            cs = cs_pool.tile([P, D + 1], BF16, tag="cs")
            nc.gpsimd.dma_start(cs[:], csd[b])
            nc.gpsimd.dma_start(
                qk[:, 2:4, :, 1024:], qkd[b][:, 2:4, :, 1024:]
            )
            return kh, kl, qh, ql, vp, cs

        def load_mask(b, ic, first=False):
            """Mask loads own the sync ring so they trigger immediately and
            prefetch ahead; out-stores ride the ACT HWDGE instead."""
            mt = mask_pool.tile([P, QT, 512], U8, tag="maskt")
            if first:
                # split the first load so the STT on key block 0 starts
                # after 256KB instead of 2MB
                for lo, hi in ((0, 4), (4, 8), (8, 16)):
                    nc.sync.dma_start(
                        mt[:, lo:hi, :], m8d[b, ic, :, lo:hi, :]
                    )
            else:
                nc.sync.dma_start(mt[:], m8d[b, ic])
            return mt

        def mm1_pair(ic, jp, kh, kl, qh, ql, mt, att):
            """scoresT + exp + mask for key blocks 2jp, 2jp+1 of chunk ic."""
            qsl = slice(ic * 512, (ic + 1) * 512)
            ps = ps_s.tile([P, 1024], F32, tag="score")
            for half in range(2):
                jb = 2 * jp + half
                ksl = slice(jb * P, (jb + 1) * P)
                osl = slice(half * 512, (half + 1) * 512)
                # q @ k ~= qh@kh + ql@kh + qh@kl, each a DoubleRow matmul
                # contracting both 128-chunks of D at 0.5 cyc/row
                nc.tensor.matmul(
                    ps[:, osl], lhsT=kh[:, :, ksl], rhs=qh[:, :, qsl],
                    start=True, stop=False, perf_mode=DR,
                )
                nc.tensor.matmul(
                    ps[:, osl], lhsT=kh[:, :, ksl], rhs=ql[:, :, qsl],
                    start=False, stop=False, perf_mode=DR,
                )
                nc.tensor.matmul(
                    ps[:, osl], lhsT=kl[:, :, ksl], rhs=qh[:, :, qsl],
                    start=False, stop=True, perf_mode=DR,
                )
            asl = att[:, 2 * jp : 2 * jp + 2, :]
            nc.scalar.activation(
                asl, ps[:], mybir.ActivationFunctionType.Exp, scale=SCALE
            )
            eng = nc.gpsimd if jp in STT_POOL_JPS else nc.vector
            eng.scalar_tensor_tensor(
                out=asl, in0=asl, scalar=-1.0, in1=mt[:, 2 * jp : 2 * jp + 2, :],
                op0=mybir.AluOpType.add, op1=mybir.AluOpType.mult,
            )

        def mm2_group(b, ic, att, vp, cs, ones, iq, last=False):
            """t.T @ [v|1|1] + colsum + normalize + store, query tile iq."""
            po = ps_out.tile([P, D + 1], F32, tag="ps_out")
            isl = slice(iq * P, (iq + 1) * P)
            for jb in range(QT):
                nc.tensor.matmul(
                    po[:], lhsT=att[:, jb, isl], rhs=vp[:, jb, :],
                    start=(jb == 0), stop=False,
                )
            # 17th block: ones/128 lhsT sums cs over partitions -> +colsum(v)
            # in cols 0..255 and +S in the Z column
            nc.tensor.matmul(po[:], lhsT=ones[:], rhs=cs[:], start=False, stop=True)
            rec = rec_pool.tile([P, 1], F32, tag="rec")
            nc.vector.reciprocal(rec[:], po[:, D : D + 1])
            osb = osb_pool.tile([P, D], BF16, tag="osb")
            if iq % 2 == 0:
                nc.scalar.activation(
                    osb[:], po[:, :D],
                    mybir.ActivationFunctionType.Copy, scale=rec[:],
                )
            else:
                nc.vector.tensor_scalar_mul(osb[:], po[:, :D], rec[:])
            it = ic * 4 + iq
            # the final chunk's stores ride the then-idle SP ring (no masks
            # left to block); earlier stores stay off it
            eng = nc.sync if last else nc.scalar
            eng.dma_start(out[b, it * P : (it + 1) * P, :], osb[:])

        batches = [b for _ in range(reps) for b in range(BPC)]
        # PE warm-up: dummy matmuls during the initial DMA wait so the clock
        # gate is at 2.4 GHz when real work arrives
        warm = one_pool.tile([P, 256], BF16, tag="warm")
        nc.vector.memset(warm[:], 0.0)
        ones = one_pool.tile([P, P], BF16, tag="ones")
        nc.vector.memset(ones[:], 1.0 / P)
        # Pool-side delay: holds the first SWDGE bulk trigger back so the
        # fast SP-ring loads win the DMA-engine FIFO in the prologue
        junk = one_pool.tile([P, 1024], BF16, tag="junk")
        nc.gpsimd.memset(junk[:], 0.0)
        for i in range(20):
            wp = ps_out.tile([P, 256], F32, tag="ps_out")
            nc.tensor.matmul(
                wp[:], lhsT=warm[:, :P], rhs=warm[:], start=True, stop=True
            )
        deferred = []
        inputs = {0: build_inputs(batches[0], first=True)}
        masks = {(0, 0): load_mask(batches[0], 0, first=True)}
        for fn in deferred:
            fn()
        pending = None

        def get_mask(idx, ic):
            if (idx, ic) not in masks:
                masks[(idx, ic)] = load_mask(batches[idx], ic)
            return masks.pop((idx, ic))

        def prefetch_mask(idx, ic):
            nxt = (idx + (ic + 1) // IC, (ic + 1) % IC)
            if nxt[0] < len(batches) and nxt not in masks:
                masks[nxt] = load_mask(batches[nxt[0]], nxt[1])

        for idx, b in enumerate(batches):
            kh, kl, qh, ql, vp, cs = inputs.pop(idx)
            for ic in range(IC):
                mt = get_mask(idx, ic)
                att = att_pool.tile([P, QT, 512], BF16, tag="att")
                for g in range(4):
                    mm1_pair(ic, 2 * g, kh, kl, qh, ql, mt, att)
                    mm1_pair(ic, 2 * g + 1, kh, kl, qh, ql, mt, att)
                    if pending is not None:
                        mm2_group(*pending, ones, iq=g)
                if ic == 1 and idx + 1 < len(batches):
                    inputs[idx + 1] = build_inputs(batches[idx + 1])
                prefetch_mask(idx, ic)
                pending = (b, ic, att, vp, cs)
        for g in range(4):
            mm2_group(*pending, ones, iq=g, last=True)

    nc.compile()
    return nc


def prep_inputs(q, k, v, mask):
    """Host-side layout prep; returns per-core in_maps."""
    q = np.asarray(q, dtype=np.float32)
    k = np.asarray(k, dtype=np.float32)
    v = np.asarray(v, dtype=np.float32)
    # [B, S, D] -> [B, P, KC, S]  (transposed, head-dim on partitions)
    qt = np.ascontiguousarray(
        q.transpose(0, 2, 1).reshape(B, KC, P, S).transpose(0, 2, 1, 3)
    )
    kt = np.ascontiguousarray(
        k.transpose(0, 2, 1).reshape(B, KC, P, S).transpose(0, 2, 1, 3)
    )
    # hi/lo fp8e4 splits: x ~= x_hi + x_lo with |err| ~ 2.5%^2
    qh = qt.astype(E4M3)
    ql = (qt - qh.astype(np.float32)).astype(E4M3)
    kh = kt.astype(E4M3)
    kl = (kt - kh.astype(np.float32)).astype(E4M3)
    # pack [B, P, 4, KC, S]: slots kh, kl, qh, ql
    qk8 = np.ascontiguousarray(
        np.stack((kh, kl, qh, ql), axis=1).transpose(0, 2, 1, 3, 4)
    )
    # [B, S, D] -> [B, P, QT, D+2] bf16 with ones in the last two columns
    vp = np.ones((B, P, QT, D + 1), dtype=NP_BF16)
    vp[..., :D] = v.reshape(B, QT, P, D).transpose(0, 2, 1, 3).astype(NP_BF16)
    # column sums of [v|1|1], replicated across partitions, pre-divided by
    # nothing: the device lhsT carries the 1/128
    csv = np.full((B, D + 1), float(S), dtype=np.float32)
    csv[:, :D] = v.sum(axis=1)
    cs = np.broadcast_to(csv[:, None, :], (B, P, D + 1)).astype(NP_BF16)
    cs = np.ascontiguousarray(cs)
    # mask [B, S(query), S(key)] -> u8 tiles [B, IC, P(key), QT, 512(query)]
    m8 = np.ascontiguousarray(
        (np.asarray(mask) != 0)
        .astype(np.uint8)
        .reshape(B, IC, 512, QT, P)
        .transpose(0, 1, 4, 3, 2)
    )
    return [
        {
            "qk8": qk8[c * BPC : (c + 1) * BPC],
            "vp": vp[c * BPC : (c + 1) * BPC],
            "cs": cs[c * BPC : (c + 1) * BPC],
            "mask8": m8[c * BPC : (c + 1) * BPC],
        }
        for c in range(NCORES)
    ]


_NC_CACHE = None


def _get_program():
    global _NC_CACHE
    if _NC_CACHE is None:
        _NC_CACHE = build_program()
    return _NC_CACHE


def postprocess(res):
    """Assemble the full f32 output from per-core results."""
    return np.concatenate(
        [
            np.asarray(res.results[c]["out"]).astype(np.float32)
            for c in range(NCORES)
        ],
        axis=0,
    )


def kernel(q, k, v, mask):
    mask = np.asarray(mask)
    if mask.sum() == 0:
        return np.zeros((B, S, D), dtype=np.float32)
    nc = _get_program()
    in_maps = prep_inputs(q, k, v, mask)
    res = run_bass_kernel_spmd(nc, in_maps, list(range(NCORES)))
    return postprocess(res)
